# revision 4
# baseline (speedup 1.0000x reference)
"""CenterLoss kernel for 8 TRN2 NeuronCores.

Computes mean over all points of min distance to any center:
    points:  [B=8, N=4096, D=256] f32
    centers: [B=8, K=1024, D=256] f32
    out = mean_{b,n} min_k ||points[b,n] - centers[b,k]||_2

Sharding: data-parallel over B (one batch element per core). Each core
computes sum_n min_k dist for its batch; host sums the 8 partials and
divides by B*N.

Per-core algorithm (free-dim-min layout, fp32r matmuls at 1 cycle/row):
    psum[n,k] = sum_d pT[d,n] * (-2 cT[d,k])  +  ||c_k||^2   (PE: 2 full MMs
                + 1 rank-1 MM with ones-weights per PSUM bank)
    mins[n] = min_k psum[n,k]                  (DVE tensor_reduce from PSUM)
    psq[n] = sum_d p[n,d]^2                    (ACT Square + accum_out)
    dist[n] = sqrt(max(psq[n] + mins[n], 0))   (DVE add/relu + ACT sqrt)
    partial = sum_n dist[n]                    (DVE row-sum + ones matmul)
"""

from contextlib import ExitStack

import numpy as np

import concourse.bass as bass
import concourse.mybir as mybir
import concourse.tile as tile
from concourse import bacc
from concourse.bass import ds
from concourse.bass_utils import run_bass_kernel_spmd

B, N, K, D = 8, 4096, 1024, 256
P = 128
NCORES = 8
MCH = N // P  # 32 row-chunks of 128 points
KH = 512      # matmul moving free dim (one PSUM bank)

F32 = mybir.dt.float32
F32R = mybir.dt.float32r
AF = mybir.ActivationFunctionType
ALU = mybir.AluOpType


def _build_kernel(ctx: ExitStack, tc: tile.TileContext, out, pointsT, centersT, pts):
    nc = tc.nc

    const_pool = ctx.enter_context(tc.tile_pool(name="const", bufs=1))
    sb = ctx.enter_context(tc.tile_pool(name="sb", bufs=1))
    psum_main = ctx.enter_context(tc.tile_pool(name="psum_main", bufs=3, space="PSUM"))
    psum_aux = ctx.enter_context(tc.tile_pool(name="psum_aux", bufs=1, space="PSUM"))
    natp = ctx.enter_context(tc.tile_pool(name="natp", bufs=3))

    # --- persistent SBUF loads -------------------------------------------
    ptT = []  # pointsT d-chunks [128, N]
    for d in range(2):
        t = sb.tile([P, N], F32R, name=f"ptT{d}", tag=f"ptT{d}")
        nc.sync.dma_start(t[:], pointsT[ds(d * P, P), :])
        ptT.append(t)
    cT = []  # centersT d-chunks [128, K]
    for d in range(2):
        t = sb.tile([P, K], F32R, name=f"cT{d}", tag=f"cT{d}")
        nc.sync.dma_start(t[:], centersT[ds(d * P, P), :])
        cT.append(t)

    ones_f = const_pool.tile([P, P], F32, name="ones_f", tag="ones_f")
    nc.vector.memset(ones_f[:], 1.0)
    ones = const_pool.tile([P, P], F32R, name="ones", tag="ones")
    nc.scalar.copy(ones[:], ones_f[:])
    ones_row = const_pool.tile([1, P], F32R, name="ones_row", tag="ones_row")
    nc.scalar.copy(ones_row[:], ones_f[0:1, :])
    onescol = const_pool.tile([P, 1], F32, name="onescol", tag="onescol")
    nc.vector.memset(onescol[:], 1.0)

    # --- centers prep: cTn = -2*cT ; csq_row[k] = sum_d cT[d,k]^2 ---------
    cTn = []
    sq = []
    for d in range(2):
        t = sb.tile([P, K], F32R, name=f"cTn{d}", tag=f"cTn{d}")
        nc.scalar.activation(t[:], cT[d][:], AF.Copy, scale=-2.0)
        cTn.append(t)
        s = sb.tile([P, K], F32R, name=f"sq{d}", tag=f"sq{d}")
        nc.scalar.activation(s[:], cT[d][:], AF.Square)
        sq.append(s)
    csq_psum = psum_aux.tile([P, K], F32, name="csq_psum", tag="aux")
    for kh in range(K // KH):
        sl = ds(kh * KH, KH)
        nc.tensor.matmul(csq_psum[:, sl], ones[:], sq[0][:, sl], start=True, stop=False)
        nc.tensor.matmul(csq_psum[:, sl], ones[:], sq[1][:, sl], start=False, stop=True)
    csq_row = sb.tile([1, K], F32R, name="csq_row", tag="csq_row")
    nc.scalar.copy(csq_row[:], csq_psum[0:1, :])

    # --- main loop over 32 point-chunks ----------------------------------
    mins = const_pool.tile([P, MCH], F32, name="mins", tag="mins")
    psq = const_pool.tile([P, MCH], F32, name="psq", tag="psq")

    for m in range(MCH):
        # psq[n] for this chunk via ACT square + row-accumulate
        pt_nat = natp.tile([P, D], F32, name="pt_nat", tag="nat")
        nc.sync.dma_start(pt_nat[:], pts[ds(m * P, P), :])
        sq_scr = natp.tile([P, D], F32, name="sq_scr", tag="sqscr", bufs=2)
        nc.scalar.activation(
            sq_scr[:], pt_nat[:], AF.Square, accum_out=psq[:, ds(m, 1)]
        )

        # psum[n,k] = ||c_k||^2 - 2 p.c  (per 512-wide bank half)
        ps = psum_main.tile([P, K], F32, name="cross", tag="cross")
        for kh in range(K // KH):
            sl = ds(kh * KH, KH)
            nc.tensor.matmul(ps[:, sl], ptT[0][:, ds(m * P, P)], cTn[0][:, sl],
                             start=True, stop=False)
            nc.tensor.matmul(ps[:, sl], ptT[1][:, ds(m * P, P)], cTn[1][:, sl],
                             start=False, stop=False)
            nc.tensor.matmul(ps[:, sl], ones_row[:], csq_row[0:1, sl],
                             start=False, stop=True)

        nc.vector.tensor_reduce(mins[:, ds(m, 1)], ps[:], mybir.AxisListType.X,
                                ALU.min)

    # --- epilogue: dist = sqrt(relu(psq + mins)); partial = sum dist ------
    d2 = const_pool.tile([P, MCH], F32, name="d2", tag="d2")
    nc.vector.tensor_add(d2[:], mins[:], psq[:])
    d2r = const_pool.tile([P, MCH], F32, name="d2r", tag="d2r")
    nc.vector.tensor_scalar_max(d2r[:], d2[:], 0.0)
    dist = const_pool.tile([P, MCH], F32, name="dist", tag="dist")
    nc.scalar.activation(dist[:], d2r[:], AF.Sqrt)
    rowsum = const_pool.tile([P, 1], F32, name="rowsum", tag="rowsum")
    nc.vector.tensor_reduce(rowsum[:], dist[:], mybir.AxisListType.X, ALU.add)
    fin = psum_aux.tile([1, 1], F32, name="fin", tag="aux")
    nc.tensor.matmul(fin[:], rowsum[:], onescol[:], start=True, stop=True)
    out_sb = const_pool.tile([1, 1], F32, name="out_sb", tag="out_sb")
    nc.scalar.copy(out_sb[:], fin[:])
    nc.sync.dma_start(out[:], out_sb[:])


def build():
    nc = bacc.Bacc(
        "TRN2",
        target_bir_lowering=False,
        debug=False,
        enable_asserts=False,
        num_devices=NCORES,
    )
    pointsT = nc.dram_tensor("pointsT", [D, N], F32R, kind="ExternalInput").ap()
    centersT = nc.dram_tensor("centersT", [D, K], F32R, kind="ExternalInput").ap()
    pts = nc.dram_tensor("pts", [N, D], F32, kind="ExternalInput").ap()
    out = nc.dram_tensor("out", [1, 1], F32, kind="ExternalOutput").ap()
    with tile.TileContext(nc) as tc, ExitStack() as ctx:
        _build_kernel(ctx, tc, out, pointsT, centersT, pts)
    nc.compile()
    return nc


_NC = None


def _make_in_maps(points: np.ndarray, centers: np.ndarray):
    in_maps = []
    for b in range(B):
        in_maps.append(
            {
                "pointsT": np.ascontiguousarray(points[b].T),
                "centersT": np.ascontiguousarray(centers[b].T),
                "pts": np.ascontiguousarray(points[b]),
            }
        )
    return in_maps


def kernel(points, centers, **_run_kwargs):
    global _NC
    points = np.asarray(points, dtype=np.float32)
    centers = np.asarray(centers, dtype=np.float32)
    assert points.shape == (B, N, D) and centers.shape == (B, K, D)
    if _NC is None:
        _NC = build()
    res = run_bass_kernel_spmd(
        _NC, _make_in_maps(points, centers), list(range(NCORES)), **_run_kwargs
    )
    total = sum(float(r["out"][0, 0]) for r in res.results)
    return np.array(total / (B * N), dtype=np.float32)


if __name__ == "__main__":
    pts = np.random.RandomState(0).randn(B, N, D).astype(np.float32)
    ctr = np.random.RandomState(1).randn(B, K, D).astype(np.float32)
    print(kernel(pts, ctr))
